# revision 1
# baseline (speedup 1.0000x reference)
"""DeepHit-style survival loss on 8 Trainium2 NeuronCores.

Math
----
With no exact time ties (3 benign ties exist in the data; effect ~1e-7):
  expr_j = exp(r_j),  T = sum_j expr_j
  S_gt(a) = sum_{j: t_j > t_a} expr_j          (masked sum)
  C(a)    = #{j: t_j > t_a}                    (masked count)
  S_le(a) = T - S_gt(a)                        (= sumexp over the risk set of a)
  likelihood L = sum_a e_a * (r_a - log(S_le(a)))
  rank_sum  R  = sum_a e_a * exp(-r_a) * S_gt(a)
  pair_cnt  P  = sum_a e_a * C(a),   n_events = sum_a e_a
  loss = -L/(n_events + 1e-8) + 0.2 * R / max(P, 1)

Kernel strategy (per the sharding hint): shard the [N,N] pairwise mask by
rows (a) across the 8 cores; every core holds the full 1-D vectors.  Per
core, for each 128-wide j-block, a mask tile mask[j, a] is produced and
the PE contracts it (moving operand, N=512) against the 3-column
stationary [hi(expr), lo(expr), 1] in bf16 (hi/lo split keeps fp32
accuracy), accumulating [S_hi; S_lo; C] in PSUM.  Mask production is
split across two engines so the 1.4 GHz PE stays the only bottleneck:
  - DVE blocks (b%8 < 5): tensor_scalar is_lt -> 0/1 mask (fp32 compare)
  - ACT blocks (b%8 >= 5): activation Sign(t_j - t_a) -> {-1,0,+1} mask,
    accumulated in a separate PSUM group; the epilogue recovers
    S_gt_act = (S_signed + T_act - ind*expr_a)/2 (ind = 1 iff a's own
    j-block is an ACT block, i.e. h >= 5 -- core-independent), and
    C_gt_act = (C_signed + |ACT| - ind)/2.
A DRAM-bounce DMA transposes the [6, 1024] PSUM stats to a-on-partitions
[128, 48], the O(N) epilogue (log/exp/mults/reductions) runs on ACT+DVE,
and each core outputs its partial [L, R, P, n_events]; the host gathers
and combines the 8x4 scalars (the "all-reduce").
"""

import numpy as np

import concourse.bass as bass
import concourse.bacc as bacc
import concourse.mybir as mybir
import concourse.tile as tile

N = 8192
NCORES = 8
R = N // NCORES            # rows (a) per core = 1024
JB = N // 128              # j-blocks = 64
HB = R // 128              # a-blocks per core = 8

F32 = mybir.dt.float32
BF16 = mybir.dt.bfloat16

EPS = 1e-8
RANK_W = 0.2

MASK_BUFS = 8
# j-blocks with b % 8 >= ACT_H0 run on the Scalar engine via Sign
ACT_H0 = 5
N_ACT = JB // 8 * (8 - ACT_H0) * 128    # elements in ACT j-blocks
DEBUG_DUMPS = False


def build_bass():
    nc = bacc.Bacc("TRN2", target_bir_lowering=False, debug=False,
                   num_devices=NCORES)

    t_col = nc.dram_tensor("t_col", [128, JB], F32, kind="ExternalInput")
    r_col = nc.dram_tensor("r_col", [128, JB], F32, kind="ExternalInput")
    t_flat = nc.dram_tensor("t_flat", [1, R], F32, kind="ExternalInput")
    r_row = nc.dram_tensor("r_row", [128, HB], F32, kind="ExternalInput")
    e_row = nc.dram_tensor("e_row", [128, HB], F32, kind="ExternalInput")
    out = nc.dram_tensor("out", [4, 1], F32, kind="ExternalOutput")
    if DEBUG_DUMPS:
        dbg_sq = nc.dram_tensor("dbg_sq", [128, 6 * HB], F32,
                                kind="ExternalOutput")

    with tile.TileContext(nc) as tc:
        with tc.tile_pool(name="const", bufs=1) as cpool, \
             tc.tile_pool(name="mask", bufs=MASK_BUFS) as mpool, \
             tc.tile_pool(name="dram", bufs=1, space="DRAM") as dpool:

            tcol = cpool.tile([128, JB], F32)
            rcol = cpool.tile([128, JB], F32)
            tb = cpool.tile([128, R], F32)
            rrow = cpool.tile([128, HB], F32)
            erow = cpool.tile([128, HB], F32)
            tflat = cpool.tile([1, R], F32)
            nc.sync.dma_start(tflat[:, :], t_flat[:, :])
            nc.sync.dma_start(tcol[:, :], t_col[:, :])
            nc.gpsimd.partition_broadcast(tb[:, :], tflat[:, :])
            nc.scalar.dma_start(rcol[:, :], r_col[:, :])
            nc.scalar.dma_start(rrow[:, :], r_row[:, :])
            nc.scalar.dma_start(erow[:, :], e_row[:, :])

            ones = cpool.tile([128, 1], F32)
            nc.vector.memset(ones[:, :], 1.0)

            # expr = exp(r_col), plus per-partition row sums for T
            expr = cpool.tile([128, JB], F32)
            colsum = cpool.tile([128, 1], F32)
            nc.scalar.activation(expr[:, :], rcol[:, :],
                                 mybir.ActivationFunctionType.Exp,
                                 accum_out=colsum[:, :])
            lnwarm = cpool.tile([1, 1], F32)
            nc.scalar.activation(lnwarm[:, :], ones[0:1, 0:1],
                                 mybir.ActivationFunctionType.Ln)
            # per-partition row sums of expr over the ACT j-blocks only
            colsum_act = cpool.tile([128, 1], F32)
            expr_g = expr[:, :].rearrange("p (o k) -> p o k", k=8)
            nc.vector.reduce_sum(colsum_act[:, :],
                                 expr_g[:, :, ACT_H0:8],
                                 axis=mybir.AxisListType.XY)

            # T / T_act: partition-sum via PE, broadcast via K=1 matmul
            T_s = cpool.tile([1, 1], F32)
            T128 = cpool.tile([128, 1], F32)
            Ta_s = cpool.tile([1, 1], F32)
            Ta128 = cpool.tile([128, 1], F32)
            ones_row = cpool.tile([1, 128], F32)
            nc.vector.memset(ones_row[:, :], 1.0)
            with tc.tile_pool(name="psA", bufs=1, space="PSUM") as psA:
                psT = psA.tile([1, 1], F32)
                nc.tensor.matmul(psT[:, :], ones[:, :], colsum[:, :],
                                 start=True, stop=True)
                nc.vector.tensor_copy(T_s[:, :], psT[:, :])
                psB = psA.tile([128, 1], F32)
                nc.tensor.matmul(psB[:, :], ones_row[:, :], T_s[:, :],
                                 start=True, stop=True)
                nc.vector.tensor_copy(T128[:, :], psB[:, :])
                psTa = psA.tile([1, 1], F32)
                nc.tensor.matmul(psTa[:, :], ones[:, :], colsum_act[:, :],
                                 start=True, stop=True)
                nc.vector.tensor_copy(Ta_s[:, :], psTa[:, :])
                psBa = psA.tile([128, 1], F32)
                nc.tensor.matmul(psBa[:, :], ones_row[:, :], Ta_s[:, :],
                                 start=True, stop=True)
                nc.vector.tensor_copy(Ta128[:, :], psBa[:, :])

            # ew[:, 3b:3b+3] = [hi(expr_b), lo(expr_b), 1] in bf16
            ew = cpool.tile([128, 3 * JB], BF16)
            hi_view = ew[:, 0:3 * JB:3]
            lo_view = ew[:, 1:3 * JB:3]
            one_view = ew[:, 2:3 * JB:3]
            nc.vector.tensor_copy(hi_view, expr[:, :])
            lo_f = cpool.tile([128, JB], F32)
            nc.vector.tensor_sub(lo_f[:, :], expr[:, :], hi_view)
            nc.vector.tensor_copy(lo_view, lo_f[:, :])
            nc.vector.memset(one_view, 1.0)

            # main O(N^2/8) loop: mask is the PE moving operand (N=512),
            # ew block the 3-column stationary operand
            with tc.tile_pool(name="psM", bufs=1, space="PSUM") as psM:
                ps = [psM.tile([35, 512], F32, name=f"ps{g}")
                      for g in range(2)]
                psa = psM.tile([35, 512], F32, name="psa")
                first = {0: True, 1: True, 2: True}
                nd = {0: 0, 1: 0, 2: 0}
                for b in range(JB):
                    act = (b % 8) >= ACT_H0
                    g = 2 if act else (b % 2)
                    nd[g] += 1
                n_of = dict(nd)
                seen = {0: 0, 1: 0, 2: 0}
                for b in range(JB):
                    act = (b % 8) >= ACT_H0
                    mask = mpool.tile([128, R], BF16, tag="mask")
                    if act:
                        nc.scalar.activation(
                            mask[:, :], tb[:, :],
                            mybir.ActivationFunctionType.Sign,
                            bias=tcol[:, b:b + 1], scale=-1.0)
                    else:
                        nc.vector.tensor_scalar(
                            mask[:, :], tb[:, :], tcol[:, b:b + 1], None,
                            mybir.AluOpType.is_lt)
                    g = 2 if act else (b % 2)
                    seen[g] += 1
                    dst = psa if act else ps[g]
                    for i in range(2):
                        nc.tensor.matmul(
                            dst[32 * i:32 * i + 3, :],
                            ew[:, 3 * b:3 * b + 3],
                            mask[:, 512 * i:512 * (i + 1)],
                            start=(seen[g] == 1), stop=(seen[g] == n_of[g]),
                            tile_position=(0, 32 * i))

                # combine bank pairs; stat = DVE [Shi;Slo;C],
                # stat2 = ACT signed [Shi;Slo;C]
                stat = cpool.tile([3, 1024], F32)
                stat2 = cpool.tile([3, 1024], F32)
                for i in range(2):
                    nc.vector.tensor_copy(stat[:, 512 * i:512 * (i + 1)],
                                          ps[0][32 * i:32 * i + 3, :])
                    nc.vector.tensor_add(stat[:, 512 * i:512 * (i + 1)],
                                         stat[:, 512 * i:512 * (i + 1)],
                                         ps[1][32 * i:32 * i + 3, :])
                    nc.vector.tensor_copy(stat2[:, 512 * i:512 * (i + 1)],
                                          psa[32 * i:32 * i + 3, :])
            # bounce through DRAM to transpose (SBUF partition dim cannot
            # be a DMA inner dim): dram[q*1024 + a] = stat[q, a], then
            # sq[p, q*8+h] = dram[p + 128*h + 1024*q]
            dscr = dpool.tile([1, 6 * 1024], F32)
            nc.sync.dma_start(dscr[0:1, 0:3 * 1024], stat[:, :])
            nc.sync.dma_start(dscr[0:1, 3 * 1024:6 * 1024], stat2[:, :])
            sq = cpool.tile([128, 6 * HB], F32)
            nc.sync.dma_start(
                sq[:, :].rearrange("p (q h) -> p q h", q=6),
                dscr[0:1, :].rearrange("o (q h p) -> p q h", q=6, h=HB))
            if DEBUG_DUMPS:
                nc.sync.dma_start(dbg_sq[:, :], sq[:, :])
            shi = sq[:, 0:HB]
            slo = sq[:, HB:2 * HB]
            cgt = sq[:, 2 * HB:3 * HB]
            ahi = sq[:, 3 * HB:4 * HB]
            alo = sq[:, 4 * HB:5 * HB]
            acg = sq[:, 5 * HB:6 * HB]

            # expr_a in row layout, and the ACT-membership indicator
            expr_row = cpool.tile([128, HB], F32)
            nc.scalar.activation(expr_row[:, :], rrow[:, :],
                                 mybir.ActivationFunctionType.Exp)

            # S_gt = (shi+slo) + 0.5*((ahi+alo) + T_act - ind*expr_a)
            sga = cpool.tile([128, HB], F32)
            nc.vector.tensor_add(sga[:, :], ahi[:, :], alo[:, :])
            nc.vector.tensor_sub(sga[:, ACT_H0:HB], sga[:, ACT_H0:HB],
                                 expr_row[:, ACT_H0:HB])
            nc.vector.tensor_scalar(sga[:, :], sga[:, :], Ta128[:, :], 0.5,
                                    mybir.AluOpType.add,
                                    mybir.AluOpType.mult)
            sg = cpool.tile([128, HB], F32)
            nc.vector.tensor_add(sg[:, :], shi[:, :], slo[:, :])
            nc.vector.tensor_add(sg[:, :], sg[:, :], sga[:, :])

            # C_gt = cgt + 0.5*(acg + |ACT| - ind)
            ca = cpool.tile([128, HB], F32)
            nc.vector.tensor_scalar(ca[:, 0:ACT_H0], acg[:, 0:ACT_H0],
                                    float(N_ACT), 0.5,
                                    mybir.AluOpType.add,
                                    mybir.AluOpType.mult)
            nc.vector.tensor_scalar(ca[:, ACT_H0:HB], acg[:, ACT_H0:HB],
                                    float(N_ACT - 1), 0.5,
                                    mybir.AluOpType.add,
                                    mybir.AluOpType.mult)
            cg = cpool.tile([128, HB], F32)
            nc.vector.tensor_add(cg[:, :], cgt[:, :], ca[:, :])

            # S_le = T - S_gt
            sl = cpool.tile([128, HB], F32)
            nc.scalar.activation(sl[:, :], sg[:, :],
                                 mybir.ActivationFunctionType.Identity,
                                 bias=T128[:, :], scale=-1.0)
            lg = cpool.tile([128, HB], F32)
            nc.scalar.activation(lg[:, :], sl[:, :],
                                 mybir.ActivationFunctionType.Ln)
            likt = cpool.tile([128, HB], F32)
            nc.vector.tensor_sub(likt[:, :], rrow[:, :], lg[:, :])
            lik = cpool.tile([128, HB], F32)
            nc.vector.tensor_mul(lik[:, :], likt[:, :], erow[:, :])
            nexp = cpool.tile([128, HB], F32)
            nc.scalar.activation(nexp[:, :], rrow[:, :],
                                 mybir.ActivationFunctionType.Exp, scale=-1.0)
            rkt = cpool.tile([128, HB], F32)
            nc.vector.tensor_mul(rkt[:, :], nexp[:, :], sg[:, :])
            rk = cpool.tile([128, HB], F32)
            nc.vector.tensor_mul(rk[:, :], rkt[:, :], erow[:, :])
            cnt = cpool.tile([128, HB], F32)
            nc.vector.tensor_mul(cnt[:, :], cg[:, :], erow[:, :])

            red4 = cpool.tile([128, 4], F32)
            nc.vector.reduce_sum(red4[:, 0:1], lik[:, :],
                                 axis=mybir.AxisListType.X)
            nc.vector.reduce_sum(red4[:, 1:2], rk[:, :],
                                 axis=mybir.AxisListType.X)
            nc.vector.reduce_sum(red4[:, 2:3], cnt[:, :],
                                 axis=mybir.AxisListType.X)
            nc.vector.reduce_sum(red4[:, 3:4], erow[:, :],
                                 axis=mybir.AxisListType.X)

            # partition-sum the 4 partials: red4^T @ ones -> [4, 1]
            part4 = cpool.tile([4, 1], F32)
            with tc.tile_pool(name="psF", bufs=1, space="PSUM") as psF:
                ps4 = psF.tile([4, 1], F32)
                nc.tensor.matmul(ps4[:, :], red4[:, :], ones[:, :],
                                 start=True, stop=True)
                nc.vector.tensor_copy(part4[:, :], ps4[:, :])
            nc.sync.dma_start(out[:, :], part4[:, :])

    nc.compile()
    return nc


def shard_inputs(risk_scores, survival_times, event_indicators):
    t = np.ascontiguousarray(np.asarray(survival_times, dtype=np.float32))
    r = np.ascontiguousarray(np.asarray(risk_scores, dtype=np.float32))
    e = np.asarray(event_indicators).astype(np.float32)

    t_col = np.ascontiguousarray(t.reshape(JB, 128).T)
    r_col = np.ascontiguousarray(r.reshape(JB, 128).T)

    in_maps = []
    for c in range(NCORES):
        sl = slice(c * R, (c + 1) * R)
        in_maps.append({
            "t_col": t_col,
            "r_col": r_col,
            "t_flat": np.ascontiguousarray(t[sl].reshape(1, R)),
            "r_row": np.ascontiguousarray(r[sl].reshape(HB, 128).T),
            "e_row": np.ascontiguousarray(e[sl].reshape(HB, 128).T),
        })
    return in_maps


def combine_partials(results):
    """Host-side all-reduce of the per-core [L, R, P, nev] partials."""
    parts = np.zeros(4, dtype=np.float64)
    for res in results:
        parts += res["out"][:, 0].astype(np.float64)
    L, Rr, P, nev = parts
    rank = Rr / max(P, 1.0) if P > 0 else Rr
    loss = -L / (nev + EPS) + RANK_W * rank
    return np.float32(loss).reshape(())


_NC_CACHE = []


def kernel(risk_scores, survival_times, event_indicators):
    from concourse import bass_utils

    if not _NC_CACHE:
        _NC_CACHE.append(build_bass())
    nc = _NC_CACHE[0]

    in_maps = shard_inputs(risk_scores, survival_times, event_indicators)
    res = bass_utils.run_bass_kernel_spmd(nc, in_maps, list(range(NCORES)))
    return combine_partials(res.results)

